# revision 1
# baseline (speedup 1.0000x reference)
"""Self-contained Trainium2 Bass kernel for nn_GCNResnet (batch-attention GCN).

Math (reference collapse):
  out[b,:] = sum_n softmax(X_n X_n^T)[b,:] @ Yh_n[:, :10] / (softmax-denom)
with Yh_n = [c_n*(X_n@W) | 1]; c_n and a constant offset fold BN(eval) +
adjacency + GCN + avgpool scalars. The softmax normalizer rides along as the
ones column of Yh: U_n = exp(S_n) @ Yh_n, out_n = U[:, :10] / U[:, 10].
(no max-subtraction: |scores| <= ~45, exp stays in fp32/bf16 range).

Sharding: row-slab parallel over 8 cores, 512 query rows per core, no
collectives. Per core per node: scores S^T chunks [128 keys x 512 rows] on the
PE (bf16 operands, 2-way row tiling on strips 0/64). The exp is the throughput
wall, so it is SPLIT between two engines working different chunk groups:
  - ACT: exp LUT, psum fp32 -> et bf16
  - DVE: Schraudolph bit-trick exp: bf16(exp(s)) ~= bitcast16(i16(s*A16+B16)),
    one tensor_scalar (mult+add, truncating int16 convert) per group
PV uses et as the *stationary* operand: U[128q, 11] += et[128k,128q]^T-op
@ xh[128k, 11] -- only 11 moving cycles per matmul, FWL bf16 weight loads.
U accumulates in one PSUM bank per node ([128, 44] = 4 q-subchunks x 11),
drained by DVE + DMA; the trivial divide/transpose happens on host.

PSUM: two scores buffers (4 banks + 3 banks) double-buffer PE vs ACT/DVE,
U accumulator 1 bank. Engine assignment alternates per group to balance
ACT (1.2 GHz) vs DVE (0.96 GHz) exp throughput.
"""

import os
import sys

if "/opt/trn_rl_repo" not in sys.path:
    sys.path.insert(0, "/opt/trn_rl_repo")

import numpy as np
import ml_dtypes

import concourse.bass as bass
import concourse.mybir as mybir
from concourse import tile
from concourse.bass_utils import run_bass_kernel_spmd
from concourse.vector_clock import ScopedClock

B, N, D = 4096, 3, 10
NCORES = 8
R = B // NCORES            # 512 query rows per core
KC = B // 128              # 32 key chunks of 128
QS = R // 128              # 4 query subchunks of 128
BN_EPS = 1e-5

# Schraudolph constants for bf16 bit patterns via int16 (truncating convert)
C16 = 9.0
A16 = float(2**7 / np.log(2))
B16 = float(127 * 128 - C16)

# schedule knobs (env-overridable for tuning sweeps)
def _envt(name, default):
    v = os.environ.get(name)
    return default if not v else tuple(
        int(x) if x.lstrip("-").isdigit() else x for x in v.split(",")
    )

GROUP_PATTERN = _envt("K_GROUPS", (2, 2, 2))   # widths, pool = gi % len
ROW_STRIPS = _envt("K_STRIPS", (0, 64))        # row-tile partition offsets
ET_BUFS = int(os.environ.get("K_ETBUFS", "2"))
PV_DELAY = int(os.environ.get("K_PVDELAY", "7"))
SLICE = int(os.environ.get("K_SLICE", "20"))
DVE_FRAC = float(os.environ.get("K_DVEFRAC", "0.5"))

# Local CoreSim (profiling) asserts every instruction carries tile-framework
# sem updates, which the walrus waitsplit workaround nops lack. The sim path
# has no walrus, so the workarounds are disabled there.
_SIM_MODE = bool(os.environ.get("KERNEL_SIM"))


def _patched_drain_and_barrier(self, tick_clock, wait_clock):
    # Walrus in this container rejects >1 sync-wait on a CTRL-class
    # instruction; absorb the tail-drain waits into SP nops, one wait each.
    nc = self.nc
    probe = nc.sync.nop()
    wait_clock.add_sem_waits(probe.ins, ScopedClock({None: tick_clock.global_clock}))
    si = probe.ins.sync_info
    waits = list(si.on_wait) if si is not None else []
    upds = list(si.on_update) if si is not None else []
    probe.ins.sync_info = mybir.SyncInfo(on_wait=waits[:1], on_update=upds)
    for w in waits[1:]:
        n = nc.sync.nop()
        n.ins.sync_info = mybir.SyncInfo(on_wait=[w], on_update=[])
    nc.sync.drain()
    nc.all_engine_barrier()
    assert self.sems is not None
    popped = nc._tile_sem_poison_stack.pop()
    assert popped is self._sem_poison
    nc.clear_and_free_semaphores(list(self.sems.allocated().values()))
    nc.all_engine_barrier()


if not _SIM_MODE:
    tile.TileContext._drain_and_barrier = _patched_drain_and_barrier

_MAX_WAITS = 1
_waitsplit_ctr = [0]


def _split_sync_waits(nc):
    """Walrus here allows very few sync-waits per instruction. Move excess
    waits onto same-engine no-ops placed immediately before the instruction
    (engine streams are in-order, so semantics are preserved)."""
    if _SIM_MODE:
        return
    for f in nc.m.functions:
        for bb in f.blocks:
            new = []
            changed = False
            for inst in bb.instructions:
                si = inst.sync_info
                waits = list(si.on_wait) if si is not None else []
                if len(waits) > _MAX_WAITS:
                    changed = True
                    for w in waits[:-_MAX_WAITS]:
                        _waitsplit_ctr[0] += 1
                        nop = mybir.InstNoOp(
                            name=f"I-waitsplit-{_waitsplit_ctr[0]}", ins=[], outs=[]
                        )
                        nop.engine = inst.engine
                        nop.sync_info = mybir.SyncInfo(on_wait=[w], on_update=[])
                        new.append(nop)
                    inst.sync_info = mybir.SyncInfo(
                        on_wait=waits[-_MAX_WAITS:], on_update=list(si.on_update)
                    )
                new.append(inst)
            if changed:
                bb.instructions = new


def _groups():
    """Chunk lists per group of one node. Pool index and engine are assigned
    from a global counter in the build loop so consecutive groups never reuse
    a pool across node/rep boundaries and the ACT/DVE ratio holds globally."""
    gs, c, gi = [], 0, 0
    while c < KC:
        w = min(GROUP_PATTERN[gi % len(GROUP_PATTERN)], KC - c)
        gs.append(list(range(c, c + w)))
        c += w
        gi += 1
    return gs


def _engine_for(g):
    return "dve" if int((g + 1) * DVE_FRAC) > int(g * DVE_FRAC) else "act"


def build_nc(rep: int = 1, rep_marker: bool = False, mode: str = "full") -> bass.Bass:
    """One-core SPMD program: full keys + this core's 512-query slab.

    mode: "full" (loads+compute per rep), "loads" (DMAs only per rep),
    "compute" (loads once, compute per rep), "nopv" (no PV/output).
    """
    f32 = mybir.dt.float32
    bf16 = mybir.dt.bfloat16
    i16 = mybir.dt.int16
    nc = bass.Bass()

    # xt: per node [10, B + R] bf16; cols 0..B-1 all keys, cols B.. this
    # core's query slab. xh: PV moving operand [128, 11] bf16 per key chunk.
    xt_d = nc.declare_dram_parameter("xt", [N, D, B + R], bf16, isOutput=False)
    xh_d = nc.declare_dram_parameter("xh", [N, KC, 128, D + 1], bf16, isOutput=False)
    UW = N * QS * (D + 1)
    uout = nc.declare_dram_parameter("uout", [128, UW + 4], f32, isOutput=True)

    groups = _groups()

    with tile.TileContext(nc) as tc:
        with (
            tc.tile_pool(name="xtp", bufs=2) as xtp,
            tc.tile_pool(name="xhp", bufs=2) as xhp,
            tc.tile_pool(name="etp", bufs=ET_BUFS) as etp,
            tc.tile_pool(name="mrk", bufs=1) as mrkp,
            tc.tile_pool(name="pssA", bufs=1, space="PSUM") as pssA,
            tc.tile_pool(name="pssB", bufs=1, space="PSUM") as pssB,
            tc.tile_pool(name="pssC", bufs=1, space="PSUM") as pssC,
            tc.tile_pool(name="psu", bufs=int(os.environ.get("K_UBUFS", "1")),
                         space="PSUM") as psu,
        ):
            ps_pools = [pssA, pssB, pssC][: len(GROUP_PATTERN)]
            gctr = [0]
            xt_sb = xh_sb = None

            def emit_pv(n_, et_, u_ps_, qs, cks):
                for ck in cks:
                    nc.tensor.matmul(
                        u_ps_[:, (D + 1) * qs : (D + 1) * (qs + 1)],
                        lhsT=et_[:, 512 * ck + 128 * qs : 512 * ck + 128 * (qs + 1)],
                        rhs=xh_sb[:].rearrange(
                            "p (n c d) -> p n c d", n=N, c=KC
                        )[:, n_, ck],
                        start=(qs == 0 and ck == 0),
                        stop=(qs == QS - 1 and ck == KC - 1),
                    )

            # Deferred-PV state: the tail of a node's qs0 sweep plus its qs1..3
            # sweeps run in slices during the NEXT node's (or rep's) early
            # groups, sharing one PSUM bank sequentially. This keeps the PE's
            # in-order stream from ever starving the exp engines, across node
            # AND rep boundaries.
            pend = {"live": False}

            def pending_slice():
                if not pend["live"]:
                    return
                qs, ck0 = pend["qs"], pend["ck"]
                ck1 = min(ck0 + SLICE, KC)
                emit_pv(pend["n"], pend["et"], pend["ups"], qs,
                        range(ck0, ck1))
                if ck1 < KC:
                    pend["ck"] = ck1
                    return
                if qs == QS - 1:
                    # whole node accumulated in one open group: one drain
                    nc.vector.tensor_copy(pend["usb"][:], pend["ups"][:])
                    nc.sync.dma_start(
                        uout[:, QS * (D + 1) * pend["n"] :
                             QS * (D + 1) * (pend["n"] + 1)],
                        pend["usb"][:],
                    )
                    pend["live"] = False
                else:
                    pend["qs"], pend["ck"] = qs + 1, 0

            for rep_i in range(rep):
                if mode != "compute" or rep_i == 0:
                    xt_sb = [
                        xtp.tile([128, B + R], bf16, tag=f"xt{n}", name=f"xt{n}")
                        for n in range(N)
                    ]
                    xh_sb = xhp.tile([128, N * KC * (D + 1)], bf16, tag="xh")
                    xh_r = xh_sb[:].rearrange("p (n c d) -> p n c d", n=N, c=KC)
                    # Load DMAs spread over the 3 DMA-capable queues so they
                    # run in parallel on HW. Node 0's three strips go first
                    # (one per queue -> ~3.6us head); later nodes use SP/Pool
                    # only, keeping ACT (the exp wall) free after the head.
                    sched = [
                        (nc.sync, 0, 0), (nc.gpsimd, 0, 1),
                        (nc.gpsimd, 0, None), (nc.sync, 1, None),
                        (nc.sync, 1, 0), (nc.gpsimd, 1, 1),
                        (nc.sync, 2, 0), (nc.gpsimd, 2, 1),
                        (nc.sync, 2, None),
                    ]
                    for eng, n, si in sched:
                        if si is None:
                            eng.dma_start(
                                xh_r[:, n], xh_d[n].rearrange("c p d -> p c d")
                            )
                        else:
                            po = ROW_STRIPS[si]
                            eng.dma_start(xt_sb[n][po : po + D, :], xt_d[n])
                if mode == "loads":
                    continue

                for n in range(N):
                    # whole-node exp(S^T) tile: 32 chunks of [128 keys, 512 q]
                    et = etp.tile([128, 512 * KC], bf16, tag="et", name="et")
                    if mode != "nopv":
                        u_sb = etp.tile([128, QS * (D + 1)], f32, tag="usb",
                                        bufs=2, name="u_sb")
                        u_ps0 = None
                    for gi, g in enumerate(groups):
                        w = len(g)
                        pidx = gctr[0] % len(ps_pools)
                        eng = _engine_for(gctr[0])
                        gctr[0] += 1
                        ps = ps_pools[pidx].tile(
                            [128, 512 * max(GROUP_PATTERN)], f32,
                            tag=f"s{pidx}", name=f"s{pidx}"
                        )
                        for i, ck in enumerate(g):
                            po = ROW_STRIPS[ck % len(ROW_STRIPS)]
                            nc.tensor.matmul(
                                ps[:, 512 * i : 512 * (i + 1)],
                                lhsT=xt_sb[n][po : po + D, 128 * ck : 128 * (ck + 1)],
                                rhs=xt_sb[n][po : po + D, B : B + R],
                                tile_position=(po, 0),
                            )
                        e_sl = et[:, 512 * g[0] : 512 * (g[0] + w)]
                        if eng == "act":
                            nc.scalar.activation(
                                e_sl,
                                ps[:, : 512 * w],
                                mybir.ActivationFunctionType.Exp,
                            )
                        else:
                            nc.vector.tensor_scalar(
                                e_sl.bitcast(i16),
                                ps[:, : 512 * w],
                                A16,
                                B16,
                                mybir.AluOpType.mult,
                                mybir.AluOpType.add,
                            )
                        if mode == "nopv":
                            continue
                        # slices of the previous node's deferred sweeps run
                        # before the inline qs0 claims the shared U bank
                        if 1 <= gi < PV_DELAY:
                            pending_slice()
                            if gi == PV_DELAY - 1:
                                while pend["live"]:
                                    pending_slice()
                        # inline qs0 PV of the group exp'd PV_DELAY groups ago
                        if gi >= PV_DELAY:
                            if u_ps0 is None:
                                u_ps0 = psu.tile([128, QS * (D + 1)], f32,
                                                 tag="u", name="u_ps0")
                            emit_pv(n, et, u_ps0, 0, groups[gi - PV_DELAY])
                    if mode == "nopv":
                        continue
                    # defer the rest of qs0 plus qs1..3 into the next node
                    resume_ck = groups[len(groups) - PV_DELAY][0]
                    if u_ps0 is None:
                        u_ps0, resume_ck = psu.tile(
                            [128, QS * (D + 1)], f32, tag="u", name="u_ps0"
                        ), 0
                    pend.update({"live": True, "n": n, "et": et, "usb": u_sb,
                                 "qs": 0, "ck": resume_ck, "ups": u_ps0})
                if rep_marker and mode != "nopv":
                    mark = mrkp.tile([1, 4], f32, tag="mark")
                    nc.vector.memset(mark[:], float(rep_i))
                    nc.sync.dma_start(uout[0:1, UW : UW + 4], mark[:])
            # final flush of the last node's deferred sweeps
            while mode not in ("loads", "nopv") and pend["live"]:
                pending_slice()
    _split_sync_waits(nc)
    return nc


def _host_prep(x, A, gc_weight, bn_gamma, bn_beta, bn_mean, bn_var):
    x = np.asarray(x, np.float32)
    A = np.asarray(A, np.float32)
    W = np.asarray(gc_weight, np.float32)
    scale = np.asarray(bn_gamma, np.float32) / np.sqrt(
        np.asarray(bn_var, np.float32) + BN_EPS
    )
    d_half = 0.5 * np.eye(N, dtype=np.float32)
    a0 = np.ones((N, N), np.float32) - np.eye(N, dtype=np.float32)
    adj = d_half @ (a0 + A) @ d_half
    wk = 0.5 * (adj[0] + adj[1])                      # [N]
    cn = (wk * scale).astype(np.float32)              # [N]
    offset = float(
        np.sum(wk * (np.asarray(bn_beta, np.float32)
                     - np.asarray(bn_mean, np.float32) * scale))
    )
    bias_vec = (offset * W.sum(axis=0)).astype(np.float32)  # [D]

    xt = np.ascontiguousarray(x.transpose(1, 2, 0))   # [N, D, B]
    xh = np.empty((N, B, D + 1), np.float32)
    for n in range(N):
        xh[n, :, :D] = (x[:, n, :] @ W) * cn[n]
        xh[n, :, D] = 1.0
    xh = np.ascontiguousarray(xh.reshape(N, KC, 128, D + 1)).astype(
        ml_dtypes.bfloat16
    )
    return xt, xh, bias_vec


def _in_maps(xt, xh):
    maps = []
    for c in range(NCORES):
        xtc = np.ascontiguousarray(
            np.concatenate([xt, xt[:, :, c * R : (c + 1) * R]], axis=2)
        ).astype(ml_dtypes.bfloat16)                   # [N, D, B + R]
        maps.append({"xt": xtc, "xh": xh})
    return maps


def _finish(uouts, bias_vec):
    """Host gather: normalize U (divide by the folded denominator column),
    sum nodes, concatenate core slabs, add the BN/adjacency bias."""
    out = np.empty((B, D), np.float32)
    for c in range(NCORES):
        u = np.asarray(uouts[c], np.float32)           # [128, N*QS*11 (+4)]
        acc = np.zeros((R, D), np.float32)
        for n in range(N):
            for qs in range(QS):
                blk = u[:, (D + 1) * (QS * n + qs) : (D + 1) * (QS * n + qs + 1)]
                acc[qs * 128 : (qs + 1) * 128] += blk[:, :D] / blk[:, D:]
        out[c * R : (c + 1) * R] = acc
    return out + bias_vec[None, :]


def kernel(**inputs) -> np.ndarray:
    assert inputs["x"].shape == (B, N, D)
    xt, xh, bias_vec = _host_prep(**inputs)
    nc = build_nc(rep=1)
    res = run_bass_kernel_spmd(nc, _in_maps(xt, xh), list(range(NCORES)))
    return _finish(
        [res.results[c]["uout"] for c in range(NCORES)], bias_vec
    ).astype(np.float32)



# revision 4
# speedup vs baseline: 1.3003x; 1.3003x over previous
"""Self-contained Trainium2 Bass kernel for nn_GCNResnet (batch-attention GCN).

Math (reference collapse):
  out[b,:] = sum_n softmax(X_n X_n^T)[b,:] @ Yh_n[:, :10] / (softmax-denom)
with Yh_n = [c_n*(X_n@W) | 1]; c_n and a constant offset fold BN(eval) +
adjacency + GCN + avgpool scalars. The softmax normalizer rides along as the
ones column of Yh: U_n = exp(S_n) @ Yh_n, out_n = U[:, :10] / U[:, 10].
(no max-subtraction: |scores| <= ~45, exp stays in fp32/bf16 range).

Sharding: row-slab parallel over 8 cores, 512 query rows per core, no
collectives. Per core, chunk-group-major over all 3 nodes: per item (g, n)
the PE computes scores S^T chunks [128 keys x 512 rows] (bf16 operands,
2-way row tiling on strips 0/64), double..triple-buffered across three
2-bank PSUM pools. The exp is the throughput wall and is SPLIT between
ACT (exp LUT) and DVE (Schraudolph bit-trick exp: bf16(exp(s)) ~=
bitcast16(i16(s*A16+B16))), alternating per item with a tunable ratio.

PV is TRANSPOSED vs the usual layout: per chunk ck and node n,
  U^T_n[11, 512] += xh_n[ck]^T @ et_n[ck]    (lhsT = xh chunk [128, 11])
so the stationary weight is only 11 columns (~9ns load) and the three
nodes' matmuls run CONCURRENTLY in separate PE column groups
(tile_position=(0, 32n)), accumulating into disjoint partition slices
(32n..32n+10) of a single PSUM bank. U banks double-buffer across reps;
the drain is a direct PSUM->DRAM DMA (no DVE/ACT involvement).
Host divides by the denominator column, sums nodes, adds the bias.
"""

import os
import sys

if "/opt/trn_rl_repo" not in sys.path:
    sys.path.insert(0, "/opt/trn_rl_repo")

import numpy as np
import ml_dtypes

import concourse.bass as bass
import concourse.mybir as mybir
from concourse import tile
from concourse.bass_utils import run_bass_kernel_spmd
from concourse.vector_clock import ScopedClock

B, N, D = 4096, 3, 10
NCORES = 8
R = B // NCORES            # 512 query rows per core
KC = B // 128              # 32 key chunks of 128
BN_EPS = 1e-5

# Schraudolph constants for bf16 bit patterns via int16 (truncating convert)
C16 = 9.0
A16 = float(2**7 / np.log(2))
B16 = float(127 * 128 - C16)

# schedule knobs (env-overridable for tuning sweeps)
GW = int(os.environ.get("K_GW", "2"))          # chunks per group
NG = KC // GW                                  # groups per node
ROW_STRIPS = tuple(
    int(x) for x in os.environ.get("K_STRIPS", "0,64").split(",")
)
PVD = int(os.environ.get("K_PVD", "2"))        # PV trails exp by PVD groups
ET_BUFS = int(os.environ.get("K_ETBUFS", "12"))
DVE_FRAC = float(os.environ.get("K_DVEFRAC", "0.4583"))
U_BUFS = int(os.environ.get("K_UBUFS", "2"))

# Local CoreSim (profiling) asserts every instruction carries tile-framework
# sem updates, which the walrus waitsplit workaround nops lack. The sim path
# has no walrus, so the workarounds are disabled there.
_SIM_MODE = bool(os.environ.get("KERNEL_SIM"))


def _patched_drain_and_barrier(self, tick_clock, wait_clock):
    # Walrus in this container rejects >1 sync-wait on a CTRL-class
    # instruction; absorb the tail-drain waits into SP nops, one wait each.
    nc = self.nc
    probe = nc.sync.nop()
    wait_clock.add_sem_waits(probe.ins, ScopedClock({None: tick_clock.global_clock}))
    si = probe.ins.sync_info
    waits = list(si.on_wait) if si is not None else []
    upds = list(si.on_update) if si is not None else []
    probe.ins.sync_info = mybir.SyncInfo(on_wait=waits[:1], on_update=upds)
    for w in waits[1:]:
        n = nc.sync.nop()
        n.ins.sync_info = mybir.SyncInfo(on_wait=[w], on_update=[])
    nc.sync.drain()
    nc.all_engine_barrier()
    assert self.sems is not None
    popped = nc._tile_sem_poison_stack.pop()
    assert popped is self._sem_poison
    nc.clear_and_free_semaphores(list(self.sems.allocated().values()))
    nc.all_engine_barrier()


if not _SIM_MODE:
    tile.TileContext._drain_and_barrier = _patched_drain_and_barrier

_MAX_WAITS = 1
_waitsplit_ctr = [0]


def _split_sync_waits(nc):
    """Walrus here allows very few sync-waits per instruction. Move excess
    waits onto same-engine no-ops placed immediately before the instruction
    (engine streams are in-order, so semantics are preserved)."""
    if _SIM_MODE:
        return
    for f in nc.m.functions:
        for bb in f.blocks:
            new = []
            changed = False
            for inst in bb.instructions:
                si = inst.sync_info
                waits = list(si.on_wait) if si is not None else []
                if len(waits) > _MAX_WAITS:
                    changed = True
                    for w in waits[:-_MAX_WAITS]:
                        _waitsplit_ctr[0] += 1
                        nop = mybir.InstNoOp(
                            name=f"I-waitsplit-{_waitsplit_ctr[0]}", ins=[], outs=[]
                        )
                        nop.engine = inst.engine
                        nop.sync_info = mybir.SyncInfo(on_wait=[w], on_update=[])
                        new.append(nop)
                    inst.sync_info = mybir.SyncInfo(
                        on_wait=waits[-_MAX_WAITS:], on_update=list(si.on_update)
                    )
                new.append(inst)
            if changed:
                bb.instructions = new


def _engine_for(i):
    return "dve" if int((i + 1) * DVE_FRAC) > int(i * DVE_FRAC) else "act"


def build_nc(rep: int = 1, rep_marker: bool = False, mode: str = "full") -> bass.Bass:
    """One-core SPMD program: full keys + this core's 512-query slab.

    mode: "full" (loads+compute per rep), "loads" (DMAs only per rep),
    "compute" (loads once, compute per rep), "nopv" (no PV/output).
    """
    f32 = mybir.dt.float32
    bf16 = mybir.dt.bfloat16
    i16 = mybir.dt.int16
    nc = bass.Bass()

    # xt: per node [10, B + R] bf16; cols 0..B-1 all keys, cols B.. this
    # core's query slab. xh: PV stationary operand [128, 11] bf16 per chunk.
    xt_d = nc.declare_dram_parameter("xt", [N, D, B + R], bf16, isOutput=False)
    xh_d = nc.declare_dram_parameter("xh", [N, KC, 128, D + 1], bf16, isOutput=False)
    uout = nc.declare_dram_parameter("uout", [N, D + 1, R + 4], f32, isOutput=True)

    nstrip = len(ROW_STRIPS)

    with tile.TileContext(nc) as tc:
        with (
            tc.tile_pool(name="xtp", bufs=2) as xtp,
            tc.tile_pool(name="xhp", bufs=2) as xhp,
            tc.tile_pool(name="etp", bufs=ET_BUFS) as etp,
            tc.tile_pool(name="mrk", bufs=1) as mrkp,
            tc.tile_pool(name="pss0", bufs=1, space="PSUM") as pss0,
            tc.tile_pool(name="pss1", bufs=1, space="PSUM") as pss1,
            tc.tile_pool(name="pss2", bufs=1, space="PSUM") as pss2,
            tc.tile_pool(name="psu", bufs=U_BUFS, space="PSUM") as psu,
        ):
            ps_pools = [pss0, pss1, pss2]
            ictr = [0]                 # global (g, n) item counter
            xt_sb = xh_sb = None
            xh_r = None

            # rolling deferred-PV queue: entries (rep_i, g, [et tiles n=0..2],
            # u_ps tile). PV for group g is emitted PVD groups later (possibly
            # in the next rep), keeping the PE from stalling on exp results
            # and letting exp of rep i+1 start while rep i's tail PV runs.
            pvq = []

            def emit_pv(ent):
                rep_i, g, ets, u_ps = ent
                for i in range(GW):
                    ck = g * GW + i
                    for n in range(N):
                        nc.tensor.matmul(
                            u_ps[32 * n : 32 * n + D + 1, :],
                            lhsT=xh_r[:, n, ck],
                            rhs=ets[n][:, 512 * i : 512 * (i + 1)],
                            start=(ck == 0),
                            stop=(ck == KC - 1),
                            tile_position=(0, 32 * n),
                        )
                if g == NG - 1:
                    # rep done: stage U^T rows 0..74 (covers the three node
                    # slices at 32n..32n+10) to SBUF in one partition-parallel
                    # copy, then DMA out. Engine alternates per rep to split
                    # the drain cost between the two exp engines.
                    u_sb = etp.tile([75, R], f32, tag="usb", bufs=2,
                                    name="u_sb")
                    if rep_i % 2 == 0:
                        nc.vector.tensor_copy(u_sb[:], u_ps[0:75, :])
                    else:
                        nc.scalar.copy(u_sb[:], u_ps[0:75, :])
                    for n in range(N):
                        nc.sync.dma_start(
                            uout[n, :, :R], u_sb[32 * n : 32 * n + D + 1, :]
                        )
                    if rep_marker:
                        mark = mrkp.tile([1, 4], f32, tag="mark")
                        nc.gpsimd.memset(mark[:], float(rep_i))
                        nc.gpsimd.dma_start(uout[0, 0, R : R + 4], mark[:])

            for rep_i in range(rep):
                if mode != "compute" or rep_i == 0:
                    xt_sb = [
                        xtp.tile([128, B + R], bf16, tag=f"xt{n}", name=f"xt{n}")
                        for n in range(N)
                    ]
                    xh_sb = xhp.tile([128, N * KC * (D + 1)], bf16, tag="xh")
                    xh_r = xh_sb[:].rearrange("p (n c d) -> p n c d", n=N, c=KC)
                    # Load DMAs spread over the 3 DMA-capable queues so they
                    # run in parallel on HW. Node 0's strips go first.
                    sched = [
                        (nc.sync, 0, 0), (nc.gpsimd, 0, 1),
                        (nc.gpsimd, 0, None), (nc.sync, 1, None),
                        (nc.sync, 1, 0), (nc.gpsimd, 1, 1),
                        (nc.sync, 2, 0), (nc.gpsimd, 2, 1),
                        (nc.sync, 2, None),
                    ]
                    for eng, n, si in sched:
                        if si is None:
                            eng.dma_start(
                                xh_r[:, n], xh_d[n].rearrange("c p d -> p c d")
                            )
                        elif si < nstrip:
                            po = ROW_STRIPS[si]
                            eng.dma_start(xt_sb[n][po : po + D, :], xt_d[n])
                if mode == "loads":
                    continue

                u_ps = None
                for g in range(NG):
                    if mode != "nopv" and u_ps is None:
                        u_ps = psu.tile([128, R], f32, tag="u", name="u_ps")
                    ets = []
                    for n in range(N):
                        it = ictr[0]
                        ictr[0] += 1
                        ps = ps_pools[it % 3].tile(
                            [128, 512 * GW], f32, tag=f"s{it % 3}",
                            name=f"s{it % 3}"
                        )
                        for i in range(GW):
                            ck = g * GW + i
                            po = ROW_STRIPS[ck % nstrip]
                            nc.tensor.matmul(
                                ps[:, 512 * i : 512 * (i + 1)],
                                lhsT=xt_sb[n][po : po + D, 128 * ck : 128 * (ck + 1)],
                                rhs=xt_sb[n][po : po + D, B : B + R],
                                tile_position=(po, 0),
                            )
                        et = etp.tile([128, 512 * GW], bf16, tag="et", name="et")
                        if _engine_for(it) == "act":
                            nc.scalar.activation(
                                et[:], ps[:], mybir.ActivationFunctionType.Exp
                            )
                        else:
                            nc.vector.tensor_scalar(
                                et[:].bitcast(i16), ps[:], A16, B16,
                                mybir.AluOpType.mult, mybir.AluOpType.add,
                            )
                        ets.append(et)
                    if mode == "nopv":
                        continue
                    pvq.append((rep_i, g, ets, u_ps))
                    if g == NG - 1:
                        u_ps = None
                    if len(pvq) > PVD:
                        emit_pv(pvq.pop(0))
            # final flush of deferred PV
            while mode not in ("loads", "nopv") and pvq:
                emit_pv(pvq.pop(0))
    _split_sync_waits(nc)
    return nc


def _host_prep(x, A, gc_weight, bn_gamma, bn_beta, bn_mean, bn_var):
    x = np.asarray(x, np.float32)
    A = np.asarray(A, np.float32)
    W = np.asarray(gc_weight, np.float32)
    scale = np.asarray(bn_gamma, np.float32) / np.sqrt(
        np.asarray(bn_var, np.float32) + BN_EPS
    )
    d_half = 0.5 * np.eye(N, dtype=np.float32)
    a0 = np.ones((N, N), np.float32) - np.eye(N, dtype=np.float32)
    adj = d_half @ (a0 + A) @ d_half
    wk = 0.5 * (adj[0] + adj[1])                      # [N]
    cn = (wk * scale).astype(np.float32)              # [N]
    offset = float(
        np.sum(wk * (np.asarray(bn_beta, np.float32)
                     - np.asarray(bn_mean, np.float32) * scale))
    )
    bias_vec = (offset * W.sum(axis=0)).astype(np.float32)  # [D]

    xt = np.ascontiguousarray(x.transpose(1, 2, 0))   # [N, D, B]
    xh = np.empty((N, B, D + 1), np.float32)
    for n in range(N):
        xh[n, :, :D] = (x[:, n, :] @ W) * cn[n]
        xh[n, :, D] = 1.0
    xh = np.ascontiguousarray(xh.reshape(N, KC, 128, D + 1)).astype(
        ml_dtypes.bfloat16
    )
    return xt, xh, bias_vec


def _in_maps(xt, xh):
    maps = []
    for c in range(NCORES):
        xtc = np.ascontiguousarray(
            np.concatenate([xt, xt[:, :, c * R : (c + 1) * R]], axis=2)
        ).astype(ml_dtypes.bfloat16)                   # [N, D, B + R]
        maps.append({"xt": xtc, "xh": xh})
    return maps


def _finish(uouts, bias_vec):
    """Host gather: normalize U^T (divide by the folded denominator row),
    sum nodes, concatenate core slabs, add the BN/adjacency bias."""
    out = np.empty((B, D), np.float32)
    for c in range(NCORES):
        u = np.asarray(uouts[c], np.float32)           # [N, 11, R (+4)]
        acc = np.zeros((R, D), np.float32)
        for n in range(N):
            acc += (u[n, :D, :R] / u[n, D : D + 1, :R]).T
        out[c * R : (c + 1) * R] = acc
    return out + bias_vec[None, :]


def kernel(**inputs) -> np.ndarray:
    assert inputs["x"].shape == (B, N, D)
    xt, xh, bias_vec = _host_prep(**inputs)
    nc = build_nc(rep=1)
    res = run_bass_kernel_spmd(nc, _in_maps(xt, xh), list(range(NCORES)))
    return _finish(
        [res.results[c]["uout"] for c in range(NCORES)], bias_vec
    ).astype(np.float32)


# revision 6
# speedup vs baseline: 2.1323x; 1.6399x over previous
"""Self-contained Trainium2 Bass kernel for nn_GCNResnet (batch-attention GCN).

Math (reference collapse):
  out[b,:] = sum_n softmax(X_n X_n^T)[b,:] @ Yh_n[:, :10] / (softmax-denom)
with Yh_n = [c_n*(X_n@W) | 1]; c_n and a constant offset fold BN(eval) +
adjacency + GCN + avgpool scalars. The softmax normalizer rides along as the
ones column of Yh: U_n = exp(S_n) @ Yh_n, out_n = U[:, :10] / U[:, 10].
(no max-subtraction: |scores| <= ~45, exp stays in fp32/bf16 range).

Sharding: row-slab parallel over 8 cores, 512 query rows per core, no
collectives. Per core, chunk-group-major over all 3 nodes: per item (g, n)
the PE computes scores S^T chunks [128 keys x 512 rows] (bf16 operands,
2-way row tiling on strips 0/64), double..triple-buffered across three
2-bank PSUM pools. The exp is the throughput wall and is SPLIT between
ACT (exp LUT) and DVE (Schraudolph bit-trick exp: bf16(exp(s)) ~=
bitcast16(i16(s*A16+B16))), alternating per item with a tunable ratio.

PV is TRANSPOSED vs the usual layout: per chunk ck and node n,
  U^T_n[11, 512] += xh_n[ck]^T @ et_n[ck]    (lhsT = xh chunk [128, 11])
so the stationary weight is only 11 columns (~9ns load) and the three
nodes' matmuls run CONCURRENTLY in separate PE column groups
(tile_position=(0, 32n)), accumulating into disjoint partition slices
(32n..32n+10) of a single PSUM bank. U banks double-buffer across reps;
the drain is a direct PSUM->DRAM DMA (no DVE/ACT involvement).
Host divides by the denominator column, sums nodes, adds the bias.
"""

import os
import sys

if "/opt/trn_rl_repo" not in sys.path:
    sys.path.insert(0, "/opt/trn_rl_repo")

import numpy as np
import ml_dtypes

import concourse.bass as bass
import concourse.mybir as mybir
from concourse import tile
from concourse.bass_utils import run_bass_kernel_spmd
from concourse.vector_clock import ScopedClock

B, N, D = 4096, 3, 10
NCORES = 8
R = B // NCORES            # 512 query rows per core
KC = B // 128              # 32 key chunks of 128
BN_EPS = 1e-5

# Schraudolph constants for bf16 bit patterns via int16 (truncating convert)
C16 = 9.0
A16 = float(2**7 / np.log(2))
B16 = float(127 * 128 - C16)

# schedule knobs (env-overridable for tuning sweeps)
GW = int(os.environ.get("K_GW", "2"))          # chunks per group
NG = KC // GW                                  # groups per node
ROW_STRIPS = tuple(
    int(x) for x in os.environ.get("K_STRIPS", "0,64").split(",")
)
PVD = int(os.environ.get("K_PVD", "2"))        # PV trails exp by PVD groups
ET_BUFS = int(os.environ.get("K_ETBUFS", "12"))
DVE_FRAC = float(os.environ.get("K_DVEFRAC", "0.4583"))
U_BUFS = int(os.environ.get("K_UBUFS", "2"))

# Local CoreSim (profiling) asserts every instruction carries tile-framework
# sem updates, which the walrus waitsplit workaround nops lack. The sim path
# has no walrus, so the workarounds are disabled there.
_SIM_MODE = bool(os.environ.get("KERNEL_SIM"))


def _patched_drain_and_barrier(self, tick_clock, wait_clock):
    # Walrus in this container rejects >1 sync-wait on a CTRL-class
    # instruction; absorb the tail-drain waits into SP nops, one wait each.
    nc = self.nc
    probe = nc.sync.nop()
    wait_clock.add_sem_waits(probe.ins, ScopedClock({None: tick_clock.global_clock}))
    si = probe.ins.sync_info
    waits = list(si.on_wait) if si is not None else []
    upds = list(si.on_update) if si is not None else []
    probe.ins.sync_info = mybir.SyncInfo(on_wait=waits[:1], on_update=upds)
    for w in waits[1:]:
        n = nc.sync.nop()
        n.ins.sync_info = mybir.SyncInfo(on_wait=[w], on_update=[])
    nc.sync.drain()
    nc.all_engine_barrier()
    assert self.sems is not None
    popped = nc._tile_sem_poison_stack.pop()
    assert popped is self._sem_poison
    nc.clear_and_free_semaphores(list(self.sems.allocated().values()))
    nc.all_engine_barrier()


if not _SIM_MODE:
    tile.TileContext._drain_and_barrier = _patched_drain_and_barrier

_MAX_WAITS = 1
_waitsplit_ctr = [0]


def _split_sync_waits(nc):
    """Walrus here allows very few sync-waits per instruction. Move excess
    waits onto same-engine no-ops placed immediately before the instruction
    (engine streams are in-order, so semantics are preserved)."""
    if _SIM_MODE:
        return
    for f in nc.m.functions:
        for bb in f.blocks:
            new = []
            changed = False
            for inst in bb.instructions:
                si = inst.sync_info
                waits = list(si.on_wait) if si is not None else []
                if len(waits) > _MAX_WAITS:
                    changed = True
                    for w in waits[:-_MAX_WAITS]:
                        _waitsplit_ctr[0] += 1
                        nop = mybir.InstNoOp(
                            name=f"I-waitsplit-{_waitsplit_ctr[0]}", ins=[], outs=[]
                        )
                        nop.engine = inst.engine
                        nop.sync_info = mybir.SyncInfo(on_wait=[w], on_update=[])
                        new.append(nop)
                    inst.sync_info = mybir.SyncInfo(
                        on_wait=waits[-_MAX_WAITS:], on_update=list(si.on_update)
                    )
                new.append(inst)
            if changed:
                bb.instructions = new


def _engine_for(i):
    return "dve" if int((i + 1) * DVE_FRAC) > int(i * DVE_FRAC) else "act"


def build_nc(rep: int = 1, rep_marker: bool = False, mode: str = "full") -> bass.Bass:
    """One-core SPMD program: full keys + this core's 512-query slab.

    mode: "full" (loads+compute per rep), "loads" (DMAs only per rep),
    "compute" (loads once, compute per rep), "nopv" (no PV/output).
    """
    f32 = mybir.dt.float32
    bf16 = mybir.dt.bfloat16
    i16 = mybir.dt.int16
    nc = bass.Bass()

    # xt: per node [10, B + R] bf16; cols 0..B-1 all keys, cols B.. this
    # core's query slab. xh: PV stationary operand [128, 11] bf16 per chunk.
    xt_d = nc.declare_dram_parameter("xt", [N, D, B + R], bf16, isOutput=False)
    xh_d = nc.declare_dram_parameter("xh", [N, KC, 128, D + 1], bf16, isOutput=False)
    uout = nc.declare_dram_parameter("uout", [N, D + 1, R + 4], f32, isOutput=True)

    nstrip = len(ROW_STRIPS)

    with tile.TileContext(nc) as tc:
        with (
            tc.tile_pool(name="xtp", bufs=2) as xtp,
            tc.tile_pool(name="xhp", bufs=2) as xhp,
            tc.tile_pool(name="etp", bufs=ET_BUFS) as etp,
            tc.tile_pool(name="mrk", bufs=1) as mrkp,
            tc.tile_pool(name="pss0", bufs=1, space="PSUM") as pss0,
            tc.tile_pool(name="pss1", bufs=1, space="PSUM") as pss1,
            tc.tile_pool(name="pss2", bufs=1, space="PSUM") as pss2,
            tc.tile_pool(name="psu", bufs=U_BUFS, space="PSUM") as psu,
        ):
            ps_pools = [pss0, pss1, pss2]
            ictr = [0]                 # global (g, n) item counter
            xt_sb = xh_sb = None
            xh_r = None

            # rolling deferred-PV queue: entries (rep_i, g, [et tiles n=0..2],
            # u_ps tile). PV for group g is emitted PVD groups later (possibly
            # in the next rep), keeping the PE from stalling on exp results
            # and letting exp of rep i+1 start while rep i's tail PV runs.
            pvq = []

            def emit_pv(ent):
                rep_i, g, ets, u_ps = ent
                for i in range(GW):
                    ck = g * GW + i
                    for n in range(N):
                        nc.tensor.matmul(
                            u_ps[32 * n : 32 * n + D + 1, :],
                            lhsT=xh_r[:, n, ck],
                            rhs=ets[n][:, 512 * i : 512 * (i + 1)],
                            start=(ck == 0),
                            stop=(ck == KC - 1),
                            tile_position=(0, 32 * n),
                        )
                if g == NG - 1:
                    # rep done: stage U^T rows 0..74 (covers the three node
                    # slices at 32n..32n+10) to SBUF in one partition-parallel
                    # copy, then DMA out. Engine alternates per rep to split
                    # the drain cost between the two exp engines.
                    u_sb = etp.tile([75, R], f32, tag="usb", bufs=2,
                                    name="u_sb")
                    if rep_i % 2 == 0:
                        nc.vector.tensor_copy(u_sb[:], u_ps[0:75, :])
                    else:
                        nc.scalar.copy(u_sb[:], u_ps[0:75, :])
                    for n in range(N):
                        nc.sync.dma_start(
                            uout[n, :, :R], u_sb[32 * n : 32 * n + D + 1, :]
                        )
                    if rep_marker:
                        mark = mrkp.tile([1, 4], f32, tag="mark")
                        nc.gpsimd.memset(mark[:], float(rep_i))
                        nc.gpsimd.dma_start(uout[0, 0, R : R + 4], mark[:])

            for rep_i in range(rep):
                if mode != "compute" or rep_i == 0:
                    xt_sb = [
                        xtp.tile([128, B + R], bf16, tag=f"xt{n}", name=f"xt{n}")
                        for n in range(N)
                    ]
                    xh_sb = xhp.tile([128, N * KC * (D + 1)], bf16, tag="xh")
                    xh_r = xh_sb[:].rearrange("p (n c d) -> p n c d", n=N, c=KC)
                    # Load DMAs spread over the 3 DMA-capable queues so they
                    # run in parallel on HW. Node 0's strips go first.
                    sched = [
                        (nc.sync, 0, 0), (nc.gpsimd, 0, 1),
                        (nc.gpsimd, 0, None), (nc.sync, 1, None),
                        (nc.sync, 1, 0), (nc.gpsimd, 1, 1),
                        (nc.sync, 2, 0), (nc.gpsimd, 2, 1),
                        (nc.sync, 2, None),
                    ]
                    for eng, n, si in sched:
                        if si is None:
                            eng.dma_start(
                                xh_r[:, n], xh_d[n].rearrange("c p d -> p c d")
                            )
                        elif si < nstrip:
                            po = ROW_STRIPS[si]
                            eng.dma_start(xt_sb[n][po : po + D, :], xt_d[n])
                if mode == "loads":
                    continue

                u_ps = None
                for g in range(NG):
                    if mode != "nopv" and u_ps is None:
                        u_ps = psu.tile([128, R], f32, tag="u", name="u_ps")
                    ets = []
                    for n in range(N):
                        it = ictr[0]
                        ictr[0] += 1
                        ps = ps_pools[it % 3].tile(
                            [128, 512 * GW], f32, tag=f"s{it % 3}",
                            name=f"s{it % 3}"
                        )
                        for i in range(GW):
                            ck = g * GW + i
                            po = ROW_STRIPS[ck % nstrip]
                            nc.tensor.matmul(
                                ps[:, 512 * i : 512 * (i + 1)],
                                lhsT=xt_sb[n][po : po + D, 128 * ck : 128 * (ck + 1)],
                                rhs=xt_sb[n][po : po + D, B : B + R],
                                tile_position=(po, 0),
                            )
                        et = etp.tile([128, 512 * GW], bf16, tag="et", name="et")
                        if _engine_for(it) == "act":
                            nc.scalar.activation(
                                et[:], ps[:], mybir.ActivationFunctionType.Exp
                            )
                        else:
                            nc.vector.tensor_scalar(
                                et[:].bitcast(i16), ps[:], A16, B16,
                                mybir.AluOpType.mult, mybir.AluOpType.add,
                            )
                        ets.append(et)
                    if mode == "nopv":
                        continue
                    pvq.append((rep_i, g, ets, u_ps))
                    if g == NG - 1:
                        u_ps = None
                    if len(pvq) > PVD:
                        emit_pv(pvq.pop(0))
            # final flush of deferred PV
            while mode not in ("loads", "nopv") and pvq:
                emit_pv(pvq.pop(0))
    _split_sync_waits(nc)
    return nc


def _host_prep(x, A, gc_weight, bn_gamma, bn_beta, bn_mean, bn_var):
    x = np.asarray(x, np.float32)
    A = np.asarray(A, np.float32)
    W = np.asarray(gc_weight, np.float32)
    scale = np.asarray(bn_gamma, np.float32) / np.sqrt(
        np.asarray(bn_var, np.float32) + BN_EPS
    )
    d_half = 0.5 * np.eye(N, dtype=np.float32)
    a0 = np.ones((N, N), np.float32) - np.eye(N, dtype=np.float32)
    adj = d_half @ (a0 + A) @ d_half
    wk = 0.5 * (adj[0] + adj[1])                      # [N]
    cn = (wk * scale).astype(np.float32)              # [N]
    offset = float(
        np.sum(wk * (np.asarray(bn_beta, np.float32)
                     - np.asarray(bn_mean, np.float32) * scale))
    )
    bias_vec = (offset * W.sum(axis=0)).astype(np.float32)  # [D]

    xt = np.ascontiguousarray(x.transpose(1, 2, 0))   # [N, D, B]
    xh = np.empty((N, B, D + 1), np.float32)
    for n in range(N):
        xh[n, :, :D] = (x[:, n, :] @ W) * cn[n]
        xh[n, :, D] = 1.0
    xh = np.ascontiguousarray(xh.reshape(N, KC, 128, D + 1)).astype(
        ml_dtypes.bfloat16
    )
    return xt, xh, bias_vec


def _in_maps(xt, xh):
    maps = []
    for c in range(NCORES):
        xtc = np.ascontiguousarray(
            np.concatenate([xt, xt[:, :, c * R : (c + 1) * R]], axis=2)
        ).astype(ml_dtypes.bfloat16)                   # [N, D, B + R]
        maps.append({"xt": xtc, "xh": xh})
    return maps


def _finish(uouts, bias_vec):
    """Host gather: normalize U^T (divide by the folded denominator row),
    sum nodes, concatenate core slabs, add the BN/adjacency bias."""
    out = np.empty((B, D), np.float32)
    for c in range(NCORES):
        u = np.asarray(uouts[c], np.float32)           # [N, 11, R (+4)]
        acc = np.zeros((R, D), np.float32)
        for n in range(N):
            acc += (u[n, :D, :R] / u[n, D : D + 1, :R]).T
        out[c * R : (c + 1) * R] = acc
    return out + bias_vec[None, :]


def _plausible(uouts) -> bool:
    """Cheap self-check against transient bad executions: every softmax
    denominator (the ones-column ride-along) must be finite and >= 1."""
    for u in uouts:
        a = np.asarray(u, np.float32)[:, : D + 1, :R]
        if not np.all(np.isfinite(a)) or not np.all(a[:, D, :] >= 0.5):
            return False
    return True


def kernel(**inputs) -> np.ndarray:
    assert inputs["x"].shape == (B, N, D)
    xt, xh, bias_vec = _host_prep(**inputs)
    nc = build_nc(rep=1)
    in_maps = _in_maps(xt, xh)
    for _attempt in range(3):
        res = run_bass_kernel_spmd(nc, in_maps, list(range(NCORES)))
        uouts = [res.results[c]["uout"] for c in range(NCORES)]
        if _plausible(uouts):
            break
    return _finish(uouts, bias_vec).astype(np.float32)
